# revision 27
# baseline (speedup 1.0000x reference)
"""Distributed Trainium2 kernel for nn_DecoderAttentionRotary.

Strategy (8 NeuronCores, tensor-parallel over heads, fp16 matmul datapath):
  - host: transpose x -> xT [D, B*L] fp16; per-core Wqkv column slice
    reordered to [q0,k0,q1,k1,v0|v1] fp16; cos/sin transposed+batch-tiled
    f32; causal masks fp16.
  - device, per core (2 heads):
      phase 1 (both batches, continuous xT stream): qkT = (Wqk^T @ xT) + b
               (fp16 matmuls, fp32 psum), v = x @ Wv directly in [l, hd]
               layout; RoPE fused per 512-column chunk (shuffle DMA + DVE)
      phase 2 (per batch, head): causal attention in scores^T layout:
               scoresT[k,q] blocks -> exp (ACT, fp16 out) -> mask (DVE 4x) ->
               out^T accum + ones-matmul rowsums (PE) ->
               normalize via gpsimd partition_broadcast + DVE
      per-batch AllToAll reshard (fp16); b=0's A2A overlaps b=1 attention.
      phase 3: y rows = outT_rows^T @ Wd + bd; Wd prefetched during
               attention (pool opened low in the SBUF stack), b=0-half
               matmuls overlap b=1 attention tail / A2A.
  - host: scatter the per-core 256-row halves into the full output.
"""
import sys

for _p in ("/opt/pypackages", "/opt/trn_rl_repo"):
    if _p not in sys.path:
        sys.path.insert(0, _p)

import numpy as np

B, L, D, H = 2, 2048, 2048, 16
HD, R = 128, 32
SCALE = float(HD) ** -0.5
W = 8
HPC = H // W              # heads per core
M = B * L                 # flattened rows
CORES = list(range(W))

_NC = None


def _build_nc():
    import concourse.mybir as mybir
    import concourse.tile as tile
    from concourse import bacc

    f32 = mybir.dt.float32
    f16 = mybir.dt.float16
    AFT = mybir.ActivationFunctionType
    OP = mybir.AluOpType

    nc = bacc.Bacc(None, target_bir_lowering=False, num_devices=W)
    xT = nc.declare_dram_parameter("xT", [D, M], f16, isOutput=False)
    wqkv = nc.declare_dram_parameter("wqkv", [D, 6 * HD], f16, isOutput=False)
    bqk = nc.declare_dram_parameter("bqk", [4 * HD, 1], f32, isOutput=False)
    bv = nc.declare_dram_parameter("bv", [1, 2 * HD], f16, isOutput=False)
    cosT = nc.declare_dram_parameter("cosT", [R, M], f16, isOutput=False)
    sinT = nc.declare_dram_parameter("sinT", [R, M], f16, isOutput=False)
    masks = nc.declare_dram_parameter("masks", [4, 128, 512], f16, isOutput=False)
    wd = nc.declare_dram_parameter("wd", [D, D], f16, isOutput=False)
    bdb = nc.declare_dram_parameter("bdb", [128, D], f32, isOutput=False)
    onesc = nc.declare_dram_parameter("onesc", [128, 1], f16, isOutput=False)
    y = nc.declare_dram_parameter("y", [M // W, D], f32, isOutput=True)

    xT_r = xT.ap().rearrange("(t p) n -> p t n", p=128)   # [128, 16, M]
    wd_r = wd.ap().rearrange("(t p) n -> p t n", p=128)

    with tile.TileContext(nc) as tc:
        with (
            tc.tile_pool(name="const", bufs=1) as cpool,
            tc.tile_pool(name="dram", bufs=1, space="DRAM") as dpool,
            tc.tile_pool(name="ps", bufs=1, space="PSUM") as pp,
            tc.tile_pool(name="qkv", bufs=1) as qkvpool,
            tc.tile_pool(name="p3", bufs=1) as p3pool,      # low in stack: prefetchable
            tc.tile_pool(name="p3s", bufs=3) as p3s,
            tc.tile_pool(name="att", bufs=3) as apool,
            tc.tile_pool(name="p1", bufs=3) as p1pool,
        ):
            warm_in = dpool.tile([W, 16], f16, name="warmin")
            warm_out = dpool.tile([W, 16], f16, name="warmout")
            a2a_ins = [dpool.tile([W, HPC * HD, 256], f16, name=f"a2ain{b}")
                       for b in range(B)]
            a2a_outs = [dpool.tile([W, HPC * HD, 256], f16, name=f"a2aout{b}")
                        for b in range(B)]

            # W load split so early k-tiles land first
            w_sb = cpool.tile([128, 16, 6 * HD], f16)
            wq_r = wqkv.ap().rearrange("(t p) m -> p t m", p=128)
            for wq in range(4):
                nc.sync.dma_start(
                    out=w_sb[:, 4 * wq:4 * (wq + 1), :],
                    in_=wq_r[:, 4 * wq:4 * (wq + 1), :],
                )
            bqk_sb = cpool.tile([128, 4], f32)
            nc.sync.dma_start(
                out=bqk_sb[:], in_=bqk.ap().rearrange("(t p) o -> p (t o)", p=128)
            )
            bv_sb = cpool.tile([1, 2 * HD], f16)
            nc.sync.dma_start(out=bv_sb[:], in_=bv.ap())
            ones_r = cpool.tile([1, 128], f16)
            nc.vector.memset(ones_r[:], 1.0)
            ones_c = cpool.tile([128, 1], f16)
            nc.sync.dma_start(out=ones_c[:], in_=onesc.ap())
            wrm = cpool.tile([1, W * 16], f16)
            nc.vector.memset(wrm[:], 0.0)
            nc.sync.dma_start(
                out=warm_in[:].rearrange("a b -> (a b)").unsqueeze(0), in_=wrm[:]
            )
            nc.gpsimd.collective_compute(
                "AllToAll",
                mybir.AluOpType.bypass,
                replica_groups=[CORES],
                ins=[warm_in[:]],
                outs=[warm_out[:]],
            )
            cos_sb = cpool.tile([R, M], f16)
            sin_sb = cpool.tile([R, M], f16)
            mask_sb = cpool.tile([128, 4, 512], f16)
            consts_loaded = False

            qk_sbs, v_sbs, o_sbs = [], [], []
            for b in range(B):
                qk_sbs.append(qkvpool.tile([128, 4, L], f16, name=f"qk{b}"))
                v_sbs.append(qkvpool.tile([128, 16, 2 * HD], f16, name=f"v{b}"))

            # ---- phase 1 (both batches) + fused RoPE ----
            for b in range(B):
                qk_sb, v_sb = qk_sbs[b], v_sbs[b]
                for nch in range(L // 512):
                    n0 = b * L + nch * 512
                    ch = slice(nch * 512, (nch + 1) * 512)
                    xt_tiles = []
                    for half in range(2):
                        xt = p1pool.tile([128, 8, 512], f16, tag="xt", bufs=3)
                        nc.sync.dma_start(
                            out=xt[:],
                            in_=xT_r[:, half * 8:(half + 1) * 8, n0:n0 + 512],
                        )
                        xt_tiles.append(xt)
                    if not consts_loaded:
                        # emitted after first xt DMAs: lower queue priority
                        nc.sync.dma_start(out=cos_sb[:], in_=cosT.ap())
                        nc.sync.dma_start(out=sin_sb[:], in_=sinT.ap())
                        nc.sync.dma_start(
                            out=mask_sb[:],
                            in_=masks.ap().rearrange("j p n -> p j n"),
                        )
                        consts_loaded = True
                    for mp in range(2):
                        pss = [
                            pp.tile([128, 512], f32, tag="work", bufs=3,
                                    name=f"qkps{b}_{nch}_{2 * mp + i}")
                            for i in range(2)
                        ]
                        for kt in range(16):
                            xt = xt_tiles[kt // 8]
                            for i in range(2):
                                m = 2 * mp + i
                                nc.tensor.matmul(
                                    pss[i][:],
                                    lhsT=w_sb[:, kt, m * 128:(m + 1) * 128],
                                    rhs=xt[:, kt % 8, :],
                                    start=(kt == 0),
                                    stop=(kt == 15),
                                )
                        for i in range(2):
                            m = 2 * mp + i
                            nc.vector.tensor_scalar_add(
                                qk_sb[:, m, ch], pss[i][:], bqk_sb[:, m:m + 1]
                            )
                    for m in range(4):
                        # fused RoPE on rows 0:R of this chunk
                        cs = cos_sb[:, n0:n0 + 512]
                        sn = sin_sb[:, n0:n0 + 512]
                        ta = p1pool.tile([R, 512], f32, tag="ta", bufs=2)
                        rot = p1pool.tile([R, 512], f16, tag="rot", bufs=2)
                        tb = p1pool.tile([R, 512], f32, tag="tb", bufs=2)
                        nc.sync.dma_start(out=rot[0:16, :], in_=qk_sb[16:32, m, ch])
                        nc.sync.dma_start(out=rot[16:32, :], in_=qk_sb[0:16, m, ch])
                        nc.vector.tensor_tensor(
                            ta[:], qk_sb[0:R, m, ch], cs, op=OP.mult
                        )
                        nc.vector.tensor_tensor(tb[:], rot[:], sn, op=OP.mult)
                        nc.vector.tensor_tensor(
                            qk_sb[0:R, m, ch], ta[:], tb[:], op=OP.add
                        )
                    for rr2 in range(2):
                        vpss = [
                            pp.tile([128, 2 * HD], f32, tag="acc", bufs=3,
                                    name=f"vps{b}_{nch}_{2 * rr2 + i}")
                            for i in range(2)
                        ]
                        for kt in range(16):
                            xt = xt_tiles[kt // 8]
                            for i in range(2):
                                rr = 2 * rr2 + i
                                nc.tensor.matmul(
                                    vpss[i][:],
                                    lhsT=xt[:, kt % 8, rr * 128:(rr + 1) * 128],
                                    rhs=w_sb[:, kt, 4 * HD:6 * HD],
                                    start=(kt == 0),
                                    stop=False,
                                )
                        for i in range(2):
                            rr = 2 * rr2 + i
                            nc.tensor.matmul(
                                vpss[i][:], lhsT=ones_r[:], rhs=bv_sb[:],
                                start=False, stop=True,
                            )
                            nc.scalar.activation(
                                v_sb[:, nch * 4 + rr, :], vpss[i][:], AFT.Copy
                            )

            # phase-3 prefetches (pool low in the stack; no deps besides a2a1)
            bd_sb = p3pool.tile([128, D], f32)
            nc.gpsimd.dma_start(out=bd_sb[:], in_=bdb.ap())
            wtiles = {}
            for bh in range(2):
                for n4 in range(4):
                    wtba = p3s.tile([128, 8, 512], f16, tag="wtba", bufs=2,
                                    name=f"wtba{bh}_{n4}")
                    nc.sync.dma_start(
                        out=wtba[:], in_=wd_r[:, 0:8, n4 * 512:(n4 + 1) * 512]
                    )
                    wtb2 = p3s.tile([128, 8, 512], f16, tag="wtb2", bufs=2,
                                    name=f"wtb2_{bh}_{n4}")
                    nc.sync.dma_start(
                        out=wtb2[:], in_=wd_r[:, 8:16, n4 * 512:(n4 + 1) * 512]
                    )
                    wtiles[(bh, n4)] = [wtba, wtb2]

            # ---- phase 2: attention, per batch; A2A after each batch ----
            for b in range(B):
                qk_sb, v_sb = qk_sbs[b], v_sbs[b]
                for h in range(HPC):
                    for qc in range(L // 512):
                        nk = 4 * qc + 4
                        outp = pp.tile([128, 512], f32, tag="acc", bufs=3,
                                       name=f"outp{b}_{h}_{qc}")
                        sump = pp.tile([1, 512], f32, tag="sump", bufs=2,
                                       name=f"sump{b}_{h}_{qc}")
                        for ki in range(nk):
                            # causal: for diagonal k-tiles only q-cols >= j*128
                            j = max(0, ki - qc * 4)
                            c0 = j * 128
                            npr = 512 - c0
                            qs = slice(qc * 512 + c0, (qc + 1) * 512)
                            sp = pp.tile([128, 512], f32, tag="work", bufs=3,
                                         name=f"sp{b}_{h}_{qc}_{ki}")
                            nc.tensor.matmul(
                                sp[:, 0:npr],
                                lhsT=qk_sb[:, 2 * h + 1, ki * 128:(ki + 1) * 128],
                                rhs=qk_sb[:, 2 * h, qs],
                                start=True, stop=True,
                            )
                            et = apool.tile([128, 512], f16, tag="et", bufs=4)
                            nc.scalar.activation(
                                et[:, 0:npr], sp[:, 0:npr], AFT.Exp, scale=SCALE
                            )
                            if ki >= qc * 4:
                                nc.vector.tensor_tensor(
                                    et[:, 0:128], et[:, 0:128],
                                    mask_sb[:, 0, 0:128], op=OP.mult,
                                )
                            nc.tensor.matmul(
                                outp[:, c0:512],
                                lhsT=v_sb[:, ki, h * 128:(h + 1) * 128],
                                rhs=et[:, 0:npr],
                                start=(ki == 0), stop=(ki == nk - 1),
                            )
                            nc.tensor.matmul(
                                sump[:, c0:512], lhsT=ones_c[:], rhs=et[:, 0:npr],
                                start=(ki == 0), stop=(ki == nk - 1),
                            )
                        osum = apool.tile([128, 512], f32, tag="osum", bufs=2)
                        nc.vector.tensor_copy(osum[:], outp[:])
                        rec = apool.tile([1, 512], f16, tag="rec", bufs=2)
                        with nc.allow_low_precision("softmax scale in fp16"):
                            nc.vector.reciprocal(rec[:], sump[:])
                        bc = pp.tile([128, 512], f32, tag="work", bufs=3,
                                     name=f"bc{b}_{h}_{qc}")
                        nc.tensor.matmul(bc[:], lhsT=ones_r[:], rhs=rec[:])
                        bcs = apool.tile([128, 512], f32, tag="bcs", bufs=2)
                        nc.scalar.activation(bcs[:], bc[:], AFT.Copy)
                        ot = apool.tile([128, 512], f16, tag="ot", bufs=2)
                        nc.vector.tensor_tensor(ot[:], osum[:], bcs[:], op=OP.mult)
                        for half in range(2):
                            nc.sync.dma_start(
                                out=a2a_ins[b][2 * qc + half,
                                               h * 128:(h + 1) * 128, :],
                                in_=ot[:, half * 256:(half + 1) * 256],
                            )
                nc.gpsimd.collective_compute(
                    "AllToAll",
                    mybir.AluOpType.bypass,
                    replica_groups=[CORES],
                    ins=[a2a_ins[b][:]],
                    outs=[a2a_outs[b][:]],
                )
                # load the resharded half + rowsums, normalize in place
                # (gpsimd/DVE are idle here; b=0's pass overlaps b=1 attention)
                o_sb = p3pool.tile([128, 16, 256], f16, name=f"osb{b}")
                o_sb_v = o_sb[:].rearrange("p (j u) n -> p j u n", j=W)
                for u in range(2):
                    nc.gpsimd.dma_start(
                        out=o_sb_v[:, :, u, :],
                        in_=a2a_outs[b][:, u * 128:(u + 1) * 128, :].rearrange(
                            "j p n -> p j n"),
                    )
                o_sbs.append(o_sb)

            # ---- phase 3: output projection, b=0 half then b=1 half ----
            for bh in range(2):
                for n4 in range(4):
                    wparts = wtiles[(bh, n4)]
                    for i in range(2):
                        m = 2 * bh + i
                        yp = pp.tile([128, 512], f32,
                                     tag=("work" if i else "acc"), bufs=3,
                                     name=f"yps{n4}_{m}")
                        for kt in range(16):
                            nc.tensor.matmul(
                                yp[:],
                                lhsT=o_sbs[bh][:, kt, i * 128:(i + 1) * 128],
                                rhs=wparts[kt // 8][:, kt % 8, :],
                                start=(kt == 0), stop=(kt == 15),
                            )
                        yt = p3s.tile([128, 512], f32, tag="yt")
                        nc.vector.tensor_tensor(
                            yt[:], yp[:], bd_sb[:, n4 * 512:(n4 + 1) * 512],
                            op=OP.add,
                        )
                        nc.sync.dma_start(
                            out=y[m * 128:(m + 1) * 128,
                                  n4 * 512:(n4 + 1) * 512],
                            in_=yt[:],
                        )
    nc.finalize()
    return nc


def _host_prep(x_BLD, cos, sin, Wqkv, bqkv, Wd, bd):
    x = np.asarray(x_BLD, np.float32).reshape(M, D)
    xT = np.ascontiguousarray(x.T.astype(np.float16))
    c2 = np.asarray(cos, np.float32).reshape(L, R).T
    s2 = np.asarray(sin, np.float32).reshape(L, R).T
    cosT = np.ascontiguousarray(np.concatenate([c2] * B, axis=1))
    sinT = np.concatenate([s2] * B, axis=1)
    sinT_pm = np.ascontiguousarray(
        np.concatenate([-sinT[:16], sinT[16:]], axis=0)
    )
    kk = np.arange(128, dtype=np.int64)[:, None]
    qq = np.arange(512, dtype=np.int64)[None, :]
    masks = np.stack(
        [(qq >= j * 128 + kk).astype(np.float16) for j in range(4)]
    )
    bdb = np.ascontiguousarray(
        np.broadcast_to(np.asarray(bd, np.float32), (128, D))
    )
    Wqkv = np.asarray(Wqkv, np.float32)
    bqkv = np.asarray(bqkv, np.float32)
    in_maps = []
    for c in range(W):
        base = c * HPC * 3 * HD
        qk_idx = np.concatenate(
            [np.arange(base + h * 3 * HD, base + h * 3 * HD + 2 * HD)
             for h in range(HPC)]
        )
        v_idx = np.concatenate(
            [np.arange(base + h * 3 * HD + 2 * HD, base + (h + 1) * 3 * HD)
             for h in range(HPC)]
        )
        in_maps.append({
            "xT": xT,
            "wqkv": np.ascontiguousarray(
                Wqkv[:, np.concatenate([qk_idx, v_idx])].astype(np.float16)
            ),
            "bqk": np.ascontiguousarray(bqkv[qk_idx].reshape(4 * HD, 1)),
            "bv": np.ascontiguousarray(
                bqkv[v_idx].reshape(1, 2 * HD).astype(np.float16)
            ),
            "cosT": cosT.astype(np.float16),
            "sinT": sinT_pm.astype(np.float16),
            "masks": masks,
            "wd": np.asarray(Wd, np.float32).astype(np.float16),
            "bdb": bdb,
            "onesc": np.ones((128, 1), np.float16),
        })
    return in_maps


def _get_nc():
    global _NC
    if _NC is None:
        _NC = _build_nc()
    return _NC


def _run(inputs, trace=False, tmpdir=None):
    from concourse.bass_utils import run_bass_kernel_spmd

    in_maps = _host_prep(**inputs)
    nc = _get_nc()
    res = run_bass_kernel_spmd(nc, in_maps, CORES, trace=trace, tmpdir=tmpdir)
    out = np.empty((M, D), np.float32)
    for c in CORES:
        yc = res.results[c]["y"]          # [512, D]: rows b0 then b1
        out[c * 256:(c + 1) * 256] = yc[:256]
        out[L + c * 256:L + (c + 1) * 256] = yc[256:]
    return out.reshape(B, L, D), res


def kernel(**inputs) -> np.ndarray:
    out, _ = _run(inputs)
    return out


# revision 29
# speedup vs baseline: 1.0693x; 1.0693x over previous
"""Distributed Trainium2 kernel for nn_DecoderAttentionRotary.

Strategy (8 NeuronCores, tensor-parallel over heads, fp16 matmul datapath):
  - host: transpose x -> xT [D, B*L] fp16; per-core Wqkv column slice
    reordered to [q0,k0,q1,k1,v0|v1] fp16; cos/sin transposed+batch-tiled
    f32; causal masks fp16.
  - device, per core (2 heads):
      phase 1 (both batches, continuous xT stream): qkT = (Wqk^T @ xT) + b
               (fp16 matmuls, fp32 psum), v = x @ Wv directly in [l, hd]
               layout; RoPE fused per 512-column chunk (shuffle DMA + DVE)
      phase 2 (per batch, head): causal attention in scores^T layout:
               scoresT[k,q] blocks -> exp (ACT, fp16 out) -> mask (DVE 4x) ->
               out^T accum + ones-matmul rowsums (PE) ->
               normalize via gpsimd partition_broadcast + DVE
      per-batch AllToAll reshard (fp16); b=0's A2A overlaps b=1 attention.
      phase 3: y rows = outT_rows^T @ Wd + bd; Wd prefetched during
               attention (pool opened low in the SBUF stack), b=0-half
               matmuls overlap b=1 attention tail / A2A.
  - host: scatter the per-core 256-row halves into the full output.
"""
import sys

for _p in ("/opt/pypackages", "/opt/trn_rl_repo"):
    if _p not in sys.path:
        sys.path.insert(0, _p)

import numpy as np

B, L, D, H = 2, 2048, 2048, 16
HD, R = 128, 32
SCALE = float(HD) ** -0.5
W = 8
HPC = H // W              # heads per core
M = B * L                 # flattened rows
CORES = list(range(W))

_NC = None


def _build_nc():
    import concourse.mybir as mybir
    import concourse.tile as tile
    from concourse import bacc

    f32 = mybir.dt.float32
    f16 = mybir.dt.float16
    AFT = mybir.ActivationFunctionType
    OP = mybir.AluOpType

    nc = bacc.Bacc(None, target_bir_lowering=False, num_devices=W)
    xT = nc.declare_dram_parameter("xT", [D, M], f16, isOutput=False)
    wqkv = nc.declare_dram_parameter("wqkv", [D, 6 * HD], f16, isOutput=False)
    bqk = nc.declare_dram_parameter("bqk", [4 * HD, 1], f32, isOutput=False)
    bv = nc.declare_dram_parameter("bv", [1, 2 * HD], f16, isOutput=False)
    cosT = nc.declare_dram_parameter("cosT", [R, M], f16, isOutput=False)
    sinT = nc.declare_dram_parameter("sinT", [R, M], f16, isOutput=False)
    masks = nc.declare_dram_parameter("masks", [4, 128, 512], f16, isOutput=False)
    wd = nc.declare_dram_parameter("wd", [D, D], f16, isOutput=False)
    bdb = nc.declare_dram_parameter("bdb", [128, D], f32, isOutput=False)
    onesc = nc.declare_dram_parameter("onesc", [128, 1], f16, isOutput=False)
    y = nc.declare_dram_parameter("y", [M // W, D], f32, isOutput=True)

    xT_r = xT.ap().rearrange("(t p) n -> p t n", p=128)   # [128, 16, M]
    wd_r = wd.ap().rearrange("(t p) n -> p t n", p=128)

    with tile.TileContext(nc) as tc:
        with (
            tc.tile_pool(name="const", bufs=1) as cpool,
            tc.tile_pool(name="dram", bufs=1, space="DRAM") as dpool,
            tc.tile_pool(name="ps", bufs=1, space="PSUM") as pp,
            tc.tile_pool(name="qkv", bufs=1) as qkvpool,
            tc.tile_pool(name="p3", bufs=1) as p3pool,      # low in stack: prefetchable
            tc.tile_pool(name="p3s", bufs=3) as p3s,
            tc.tile_pool(name="att", bufs=3) as apool,
            tc.tile_pool(name="p1", bufs=3) as p1pool,
        ):
            a2a_ins = [dpool.tile([W, HPC * HD, 256], f16, name=f"a2ain{b}")
                       for b in range(B)]
            a2a_outs = [dpool.tile([W, HPC * HD, 256], f16, name=f"a2aout{b}")
                        for b in range(B)]

            # W load split so early k-tiles land first
            w_sb = cpool.tile([128, 16, 6 * HD], f16)
            wq_r = wqkv.ap().rearrange("(t p) m -> p t m", p=128)
            for wq in range(4):
                nc.sync.dma_start(
                    out=w_sb[:, 4 * wq:4 * (wq + 1), :],
                    in_=wq_r[:, 4 * wq:4 * (wq + 1), :],
                )
            bqk_sb = cpool.tile([128, 4], f32)
            nc.sync.dma_start(
                out=bqk_sb[:], in_=bqk.ap().rearrange("(t p) o -> p (t o)", p=128)
            )
            bv_sb = cpool.tile([1, 2 * HD], f16)
            nc.sync.dma_start(out=bv_sb[:], in_=bv.ap())
            ones_r = cpool.tile([1, 128], f16)
            nc.vector.memset(ones_r[:], 1.0)
            ones_c = cpool.tile([128, 1], f16)
            nc.sync.dma_start(out=ones_c[:], in_=onesc.ap())
            cos_sb = cpool.tile([R, M], f16)
            sin_sb = cpool.tile([R, M], f16)
            mask_sb = cpool.tile([128, 4, 512], f16)
            consts_loaded = False

            qk_sbs, v_sbs, o_sbs = [], [], []
            for b in range(B):
                qk_sbs.append(qkvpool.tile([128, 4, L], f16, name=f"qk{b}"))
                v_sbs.append(qkvpool.tile([128, 16, 2 * HD], f16, name=f"v{b}"))

            # ---- phase 1 (both batches) + fused RoPE ----
            for b in range(B):
                qk_sb, v_sb = qk_sbs[b], v_sbs[b]
                for nch in range(L // 512):
                    n0 = b * L + nch * 512
                    ch = slice(nch * 512, (nch + 1) * 512)
                    xt_tiles = []
                    for half in range(2):
                        xt = p1pool.tile([128, 8, 512], f16, tag="xt", bufs=3)
                        nc.sync.dma_start(
                            out=xt[:],
                            in_=xT_r[:, half * 8:(half + 1) * 8, n0:n0 + 512],
                        )
                        xt_tiles.append(xt)
                    if not consts_loaded:
                        # emitted after first xt DMAs: lower queue priority
                        nc.sync.dma_start(out=cos_sb[:], in_=cosT.ap())
                        nc.sync.dma_start(out=sin_sb[:], in_=sinT.ap())
                        nc.sync.dma_start(
                            out=mask_sb[:],
                            in_=masks.ap().rearrange("j p n -> p j n"),
                        )
                        consts_loaded = True
                    for mp in range(2):
                        pss = [
                            pp.tile([128, 512], f32, tag="work", bufs=3,
                                    name=f"qkps{b}_{nch}_{2 * mp + i}")
                            for i in range(2)
                        ]
                        for kt in range(16):
                            xt = xt_tiles[kt // 8]
                            for i in range(2):
                                m = 2 * mp + i
                                nc.tensor.matmul(
                                    pss[i][:],
                                    lhsT=w_sb[:, kt, m * 128:(m + 1) * 128],
                                    rhs=xt[:, kt % 8, :],
                                    start=(kt == 0),
                                    stop=(kt == 15),
                                )
                        for i in range(2):
                            m = 2 * mp + i
                            nc.vector.tensor_scalar_add(
                                qk_sb[:, m, ch], pss[i][:], bqk_sb[:, m:m + 1]
                            )
                    for m in range(4):
                        # fused RoPE on rows 0:R of this chunk
                        cs = cos_sb[:, n0:n0 + 512]
                        sn = sin_sb[:, n0:n0 + 512]
                        ta = p1pool.tile([R, 512], f32, tag="ta", bufs=2)
                        rot = p1pool.tile([R, 512], f16, tag="rot", bufs=2)
                        tb = p1pool.tile([R, 512], f32, tag="tb", bufs=2)
                        nc.sync.dma_start(out=rot[0:16, :], in_=qk_sb[16:32, m, ch])
                        nc.sync.dma_start(out=rot[16:32, :], in_=qk_sb[0:16, m, ch])
                        nc.vector.tensor_tensor(
                            ta[:], qk_sb[0:R, m, ch], cs, op=OP.mult
                        )
                        nc.vector.tensor_tensor(tb[:], rot[:], sn, op=OP.mult)
                        nc.vector.tensor_tensor(
                            qk_sb[0:R, m, ch], ta[:], tb[:], op=OP.add
                        )
                    for rr2 in range(2):
                        vpss = [
                            pp.tile([128, 2 * HD], f32, tag="acc", bufs=3,
                                    name=f"vps{b}_{nch}_{2 * rr2 + i}")
                            for i in range(2)
                        ]
                        for kt in range(16):
                            xt = xt_tiles[kt // 8]
                            for i in range(2):
                                rr = 2 * rr2 + i
                                nc.tensor.matmul(
                                    vpss[i][:],
                                    lhsT=xt[:, kt % 8, rr * 128:(rr + 1) * 128],
                                    rhs=w_sb[:, kt, 4 * HD:6 * HD],
                                    start=(kt == 0),
                                    stop=False,
                                )
                        for i in range(2):
                            rr = 2 * rr2 + i
                            nc.tensor.matmul(
                                vpss[i][:], lhsT=ones_r[:], rhs=bv_sb[:],
                                start=False, stop=True,
                            )
                            nc.scalar.activation(
                                v_sb[:, nch * 4 + rr, :], vpss[i][:], AFT.Copy
                            )

            # phase-3 prefetches (pool low in the stack; no deps besides a2a1)
            bd_sb = p3pool.tile([128, D], f32)
            nc.gpsimd.dma_start(out=bd_sb[:], in_=bdb.ap())
            wtiles = {}
            for bh in range(2):
                for n4 in range(4):
                    wtba = p3s.tile([128, 8, 512], f16, tag="wtba", bufs=2,
                                    name=f"wtba{bh}_{n4}")
                    nc.sync.dma_start(
                        out=wtba[:], in_=wd_r[:, 0:8, n4 * 512:(n4 + 1) * 512]
                    )
                    wtb2 = p3s.tile([128, 8, 512], f16, tag="wtb2", bufs=2,
                                    name=f"wtb2_{bh}_{n4}")
                    nc.sync.dma_start(
                        out=wtb2[:], in_=wd_r[:, 8:16, n4 * 512:(n4 + 1) * 512]
                    )
                    wtiles[(bh, n4)] = [wtba, wtb2]

            # ---- phase 2: attention, per batch; A2A after each batch ----
            for b in range(B):
                qk_sb, v_sb = qk_sbs[b], v_sbs[b]
                for h in range(HPC):
                    for qc in range(L // 512):
                        nk = 4 * qc + 4
                        outp = pp.tile([128, 512], f32, tag="acc", bufs=3,
                                       name=f"outp{b}_{h}_{qc}")
                        sump = pp.tile([1, 512], f32, tag="sump", bufs=2,
                                       name=f"sump{b}_{h}_{qc}")
                        for ki in range(nk):
                            # causal: for diagonal k-tiles only q-cols >= j*128
                            j = max(0, ki - qc * 4)
                            c0 = j * 128
                            npr = 512 - c0
                            qs = slice(qc * 512 + c0, (qc + 1) * 512)
                            sp = pp.tile([128, 512], f32, tag="work", bufs=3,
                                         name=f"sp{b}_{h}_{qc}_{ki}")
                            nc.tensor.matmul(
                                sp[:, 0:npr],
                                lhsT=qk_sb[:, 2 * h + 1, ki * 128:(ki + 1) * 128],
                                rhs=qk_sb[:, 2 * h, qs],
                                start=True, stop=True,
                            )
                            et = apool.tile([128, 512], f16, tag="et", bufs=4)
                            nc.scalar.activation(
                                et[:, 0:npr], sp[:, 0:npr], AFT.Exp, scale=SCALE
                            )
                            if ki >= qc * 4:
                                nc.vector.tensor_tensor(
                                    et[:, 0:128], et[:, 0:128],
                                    mask_sb[:, 0, 0:128], op=OP.mult,
                                )
                            nc.tensor.matmul(
                                outp[:, c0:512],
                                lhsT=v_sb[:, ki, h * 128:(h + 1) * 128],
                                rhs=et[:, 0:npr],
                                start=(ki == 0), stop=(ki == nk - 1),
                            )
                            nc.tensor.matmul(
                                sump[:, c0:512], lhsT=ones_c[:], rhs=et[:, 0:npr],
                                start=(ki == 0), stop=(ki == nk - 1),
                            )
                        ot = apool.tile([128, 512], f16, tag="ot", bufs=3)
                        nc.vector.tensor_copy(ot[:], outp[:])
                        rec = apool.tile([1, 512], f16, tag="rec", bufs=2)
                        with nc.allow_low_precision("softmax scale in fp16"):
                            nc.vector.reciprocal(rec[:], sump[:])
                        bc = pp.tile([128, 512], f32, tag="sump", bufs=2,
                                     name=f"bc{b}_{h}_{qc}")
                        nc.tensor.matmul(bc[:], lhsT=ones_r[:], rhs=rec[:])
                        nc.vector.tensor_tensor(ot[:], ot[:], bc[:], op=OP.mult)
                        for half in range(2):
                            nc.sync.dma_start(
                                out=a2a_ins[b][2 * qc + half,
                                               h * 128:(h + 1) * 128, :],
                                in_=ot[:, half * 256:(half + 1) * 256],
                            )
                nc.gpsimd.collective_compute(
                    "AllToAll",
                    mybir.AluOpType.bypass,
                    replica_groups=[CORES],
                    ins=[a2a_ins[b][:]],
                    outs=[a2a_outs[b][:]],
                )
                # load the resharded half + rowsums, normalize in place
                # (gpsimd/DVE are idle here; b=0's pass overlaps b=1 attention)
                o_sb = p3pool.tile([128, 16, 256], f16, name=f"osb{b}")
                o_sb_v = o_sb[:].rearrange("p (j u) n -> p j u n", j=W)
                for u in range(2):
                    nc.gpsimd.dma_start(
                        out=o_sb_v[:, :, u, :],
                        in_=a2a_outs[b][:, u * 128:(u + 1) * 128, :].rearrange(
                            "j p n -> p j n"),
                    )
                o_sbs.append(o_sb)

            # ---- phase 3: output projection, b=0 half then b=1 half ----
            for bh in range(2):
                for n4 in range(4):
                    wparts = wtiles[(bh, n4)]
                    for i in range(2):
                        m = 2 * bh + i
                        yp = pp.tile([128, 512], f32,
                                     tag=("work" if i else "acc"), bufs=3,
                                     name=f"yps{n4}_{m}")
                        for kt in range(16):
                            nc.tensor.matmul(
                                yp[:],
                                lhsT=o_sbs[bh][:, kt, i * 128:(i + 1) * 128],
                                rhs=wparts[kt // 8][:, kt % 8, :],
                                start=(kt == 0), stop=(kt == 15),
                            )
                        yt = p3s.tile([128, 512], f32, tag="yt")
                        nc.vector.tensor_tensor(
                            yt[:], yp[:], bd_sb[:, n4 * 512:(n4 + 1) * 512],
                            op=OP.add,
                        )
                        nc.sync.dma_start(
                            out=y[m * 128:(m + 1) * 128,
                                  n4 * 512:(n4 + 1) * 512],
                            in_=yt[:],
                        )
    nc.finalize()
    return nc


def _host_prep(x_BLD, cos, sin, Wqkv, bqkv, Wd, bd):
    x = np.asarray(x_BLD, np.float32).reshape(M, D)
    xT = np.ascontiguousarray(x.T.astype(np.float16))
    c2 = np.asarray(cos, np.float32).reshape(L, R).T
    s2 = np.asarray(sin, np.float32).reshape(L, R).T
    cosT = np.ascontiguousarray(np.concatenate([c2] * B, axis=1))
    sinT = np.concatenate([s2] * B, axis=1)
    sinT_pm = np.ascontiguousarray(
        np.concatenate([-sinT[:16], sinT[16:]], axis=0)
    )
    kk = np.arange(128, dtype=np.int64)[:, None]
    qq = np.arange(512, dtype=np.int64)[None, :]
    masks = np.stack(
        [(qq >= j * 128 + kk).astype(np.float16) for j in range(4)]
    )
    bdb = np.ascontiguousarray(
        np.broadcast_to(np.asarray(bd, np.float32), (128, D))
    )
    Wqkv = np.asarray(Wqkv, np.float32)
    bqkv = np.asarray(bqkv, np.float32)
    in_maps = []
    for c in range(W):
        base = c * HPC * 3 * HD
        qk_idx = np.concatenate(
            [np.arange(base + h * 3 * HD, base + h * 3 * HD + 2 * HD)
             for h in range(HPC)]
        )
        v_idx = np.concatenate(
            [np.arange(base + h * 3 * HD + 2 * HD, base + (h + 1) * 3 * HD)
             for h in range(HPC)]
        )
        in_maps.append({
            "xT": xT,
            "wqkv": np.ascontiguousarray(
                Wqkv[:, np.concatenate([qk_idx, v_idx])].astype(np.float16)
            ),
            "bqk": np.ascontiguousarray(bqkv[qk_idx].reshape(4 * HD, 1)),
            "bv": np.ascontiguousarray(
                bqkv[v_idx].reshape(1, 2 * HD).astype(np.float16)
            ),
            "cosT": cosT.astype(np.float16),
            "sinT": sinT_pm.astype(np.float16),
            "masks": masks,
            "wd": np.asarray(Wd, np.float32).astype(np.float16),
            "bdb": bdb,
            "onesc": np.ones((128, 1), np.float16),
        })
    return in_maps


def _get_nc():
    global _NC
    if _NC is None:
        _NC = _build_nc()
    return _NC


def _run(inputs, trace=False, tmpdir=None):
    from concourse.bass_utils import run_bass_kernel_spmd

    in_maps = _host_prep(**inputs)
    nc = _get_nc()
    res = run_bass_kernel_spmd(nc, in_maps, CORES, trace=trace, tmpdir=tmpdir)
    out = np.empty((M, D), np.float32)
    for c in CORES:
        yc = res.results[c]["y"]          # [512, D]: rows b0 then b1
        out[c * 256:(c + 1) * 256] = yc[:256]
        out[L + c * 256:L + (c + 1) * 256] = yc[256:]
    return out.reshape(B, L, D), res


def kernel(**inputs) -> np.ndarray:
    out, _ = _run(inputs)
    return out


# revision 31
# speedup vs baseline: 1.1043x; 1.0328x over previous
"""Distributed Trainium2 kernel for nn_DecoderAttentionRotary.

Strategy (8 NeuronCores, tensor-parallel over heads, fp16 matmul datapath):
  - host: transpose x -> xT [D, B*L] fp16; per-core Wqkv column slice
    reordered to [q0,k0,q1,k1,v0|v1] fp16; cos/sin transposed+batch-tiled
    f32; causal masks fp16.
  - device, per core (2 heads):
      phase 1 (both batches, continuous xT stream): qkT = (Wqk^T @ xT) + b
               (fp16 matmuls, fp32 psum), v = x @ Wv directly in [l, hd]
               layout; RoPE fused per 512-column chunk (shuffle DMA + DVE)
      phase 2 (per batch, head): causal attention in scores^T layout:
               scoresT[k,q] blocks -> exp (ACT, fp16 out) -> mask (DVE 4x) ->
               out^T accum + ones-matmul rowsums (PE) ->
               normalize via gpsimd partition_broadcast + DVE
      per-batch AllToAll reshard (fp16); b=0's A2A overlaps b=1 attention.
      phase 3: y rows = outT_rows^T @ Wd + bd; Wd prefetched during
               attention (pool opened low in the SBUF stack), b=0-half
               matmuls overlap b=1 attention tail / A2A.
  - host: scatter the per-core 256-row halves into the full output.
"""
import sys

for _p in ("/opt/pypackages", "/opt/trn_rl_repo"):
    if _p not in sys.path:
        sys.path.insert(0, _p)

import numpy as np

B, L, D, H = 2, 2048, 2048, 16
HD, R = 128, 32
SCALE = float(HD) ** -0.5
W = 8
HPC = H // W              # heads per core
M = B * L                 # flattened rows
CORES = list(range(W))

_NC = None


def _build_nc():
    import concourse.mybir as mybir
    import concourse.tile as tile
    from concourse import bacc

    f32 = mybir.dt.float32
    f16 = mybir.dt.float16
    AFT = mybir.ActivationFunctionType
    OP = mybir.AluOpType

    nc = bacc.Bacc(None, target_bir_lowering=False, num_devices=W)
    xT = nc.declare_dram_parameter("xT", [D, M], f16, isOutput=False)
    wqkv = nc.declare_dram_parameter("wqkv", [D, 6 * HD], f16, isOutput=False)
    bqk = nc.declare_dram_parameter("bqk", [4 * HD, 1], f32, isOutput=False)
    bv = nc.declare_dram_parameter("bv", [1, 2 * HD], f16, isOutput=False)
    cosT = nc.declare_dram_parameter("cosT", [R, M], f16, isOutput=False)
    sinT = nc.declare_dram_parameter("sinT", [R, M], f16, isOutput=False)
    masks = nc.declare_dram_parameter("masks", [4, 128, 512], f16, isOutput=False)
    wd = nc.declare_dram_parameter("wd", [D, D], f16, isOutput=False)
    bdb = nc.declare_dram_parameter("bdb", [128, D], f32, isOutput=False)
    onesc = nc.declare_dram_parameter("onesc", [128, 1], f16, isOutput=False)
    y = nc.declare_dram_parameter("y", [M // W, D], f32, isOutput=True)

    xT_r = xT.ap().rearrange("(t p) n -> p t n", p=128)   # [128, 16, M]
    wd_r = wd.ap().rearrange("(t p) n -> p t n", p=128)

    with tile.TileContext(nc) as tc:
        with (
            tc.tile_pool(name="const", bufs=1) as cpool,
            tc.tile_pool(name="dram", bufs=1, space="DRAM") as dpool,
            tc.tile_pool(name="ps", bufs=1, space="PSUM") as pp,
            tc.tile_pool(name="qkv", bufs=1) as qkvpool,
            tc.tile_pool(name="p3", bufs=1) as p3pool,      # low in stack: prefetchable
            tc.tile_pool(name="p3s", bufs=3) as p3s,
            tc.tile_pool(name="att", bufs=3) as apool,
            tc.tile_pool(name="p1", bufs=3) as p1pool,
        ):
            a2a_ins = [dpool.tile([W, HPC * HD, 256], f16, name=f"a2ain{b}")
                       for b in range(B)]
            a2a_outs = [dpool.tile([W, HPC * HD, 256], f16, name=f"a2aout{b}")
                        for b in range(B)]

            # W load split so early k-tiles land first
            w_sb = cpool.tile([128, 16, 6 * HD], f16)
            wq_r = wqkv.ap().rearrange("(t p) m -> p t m", p=128)
            for wq in range(4):
                nc.sync.dma_start(
                    out=w_sb[:, 4 * wq:4 * (wq + 1), :],
                    in_=wq_r[:, 4 * wq:4 * (wq + 1), :],
                )
            bqk_sb = cpool.tile([128, 4], f32)
            nc.sync.dma_start(
                out=bqk_sb[:], in_=bqk.ap().rearrange("(t p) o -> p (t o)", p=128)
            )
            bv_sb = cpool.tile([1, 2 * HD], f16)
            nc.sync.dma_start(out=bv_sb[:], in_=bv.ap())
            ones_r = cpool.tile([1, 128], f16)
            nc.vector.memset(ones_r[:], 1.0)
            ones_c = cpool.tile([128, 1], f16)
            nc.sync.dma_start(out=ones_c[:], in_=onesc.ap())
            cos_sb = cpool.tile([R, M], f16)
            sin_sb = cpool.tile([R, M], f16)
            mask_sb = cpool.tile([128, 4, 512], f16)
            consts_loaded = False

            qk_sbs, v_sbs, o_sbs = [], [], []
            for b in range(B):
                qk_sbs.append(qkvpool.tile([128, 4, L], f16, name=f"qk{b}"))
                v_sbs.append(qkvpool.tile([128, 16, 2 * HD], f16, name=f"v{b}"))

            # ---- phase 1 (both batches) + fused RoPE ----
            for b in range(B):
                qk_sb, v_sb = qk_sbs[b], v_sbs[b]
                for nch in range(L // 512):
                    n0 = b * L + nch * 512
                    ch = slice(nch * 512, (nch + 1) * 512)
                    xt_tiles = []
                    for half in range(2):
                        xt = p1pool.tile([128, 8, 512], f16, tag="xt", bufs=3)
                        nc.sync.dma_start(
                            out=xt[:],
                            in_=xT_r[:, half * 8:(half + 1) * 8, n0:n0 + 512],
                        )
                        xt_tiles.append(xt)
                    if not consts_loaded:
                        # emitted after first xt DMAs: lower queue priority
                        nc.sync.dma_start(out=cos_sb[:], in_=cosT.ap())
                        nc.sync.dma_start(out=sin_sb[:], in_=sinT.ap())
                        nc.sync.dma_start(
                            out=mask_sb[:],
                            in_=masks.ap().rearrange("j p n -> p j n"),
                        )
                        consts_loaded = True
                    for mp in range(2):
                        pss = [
                            pp.tile([128, 512], f32, tag="work", bufs=3,
                                    name=f"qkps{b}_{nch}_{2 * mp + i}")
                            for i in range(2)
                        ]
                        for kt in range(16):
                            xt = xt_tiles[kt // 8]
                            for i in range(2):
                                m = 2 * mp + i
                                nc.tensor.matmul(
                                    pss[i][:],
                                    lhsT=w_sb[:, kt, m * 128:(m + 1) * 128],
                                    rhs=xt[:, kt % 8, :],
                                    start=(kt == 0),
                                    stop=(kt == 15),
                                )
                        for i in range(2):
                            m = 2 * mp + i
                            nc.vector.tensor_scalar_add(
                                qk_sb[:, m, ch], pss[i][:], bqk_sb[:, m:m + 1]
                            )
                    for m in range(4):
                        # fused RoPE on rows 0:R of this chunk
                        cs = cos_sb[:, n0:n0 + 512]
                        sn = sin_sb[:, n0:n0 + 512]
                        ta = p1pool.tile([R, 512], f32, tag="ta", bufs=2)
                        rot = p1pool.tile([R, 512], f16, tag="rot", bufs=2)
                        tb = p1pool.tile([R, 512], f32, tag="tb", bufs=2)
                        nc.sync.dma_start(out=rot[0:16, :], in_=qk_sb[16:32, m, ch])
                        nc.sync.dma_start(out=rot[16:32, :], in_=qk_sb[0:16, m, ch])
                        nc.vector.tensor_tensor(
                            ta[:], qk_sb[0:R, m, ch], cs, op=OP.mult
                        )
                        nc.vector.tensor_tensor(tb[:], rot[:], sn, op=OP.mult)
                        nc.vector.tensor_tensor(
                            qk_sb[0:R, m, ch], ta[:], tb[:], op=OP.add
                        )
                    for rr2 in range(2):
                        vpss = [
                            pp.tile([128, 2 * HD], f32, tag="acc", bufs=3,
                                    name=f"vps{b}_{nch}_{2 * rr2 + i}")
                            for i in range(2)
                        ]
                        for kt in range(16):
                            xt = xt_tiles[kt // 8]
                            for i in range(2):
                                rr = 2 * rr2 + i
                                nc.tensor.matmul(
                                    vpss[i][:],
                                    lhsT=xt[:, kt % 8, rr * 128:(rr + 1) * 128],
                                    rhs=w_sb[:, kt, 4 * HD:6 * HD],
                                    start=(kt == 0),
                                    stop=False,
                                )
                        for i in range(2):
                            rr = 2 * rr2 + i
                            nc.tensor.matmul(
                                vpss[i][:], lhsT=ones_r[:], rhs=bv_sb[:],
                                start=False, stop=True,
                            )
                            nc.scalar.activation(
                                v_sb[:, nch * 4 + rr, :], vpss[i][:], AFT.Copy
                            )

            # phase-3 prefetches (pool low in the stack; no deps besides a2a1)
            bd_sb = p3pool.tile([128, D], f32)
            nc.gpsimd.dma_start(out=bd_sb[:], in_=bdb.ap())
            wtiles = {}
            for bh in range(2):
                for n4 in range(4):
                    wtba = p3s.tile([128, 8, 512], f16, tag="wtba", bufs=2,
                                    name=f"wtba{bh}_{n4}")
                    nc.sync.dma_start(
                        out=wtba[:], in_=wd_r[:, 0:8, n4 * 512:(n4 + 1) * 512]
                    )
                    wtb2 = p3s.tile([128, 8, 512], f16, tag="wtb2", bufs=2,
                                    name=f"wtb2_{bh}_{n4}")
                    nc.sync.dma_start(
                        out=wtb2[:], in_=wd_r[:, 8:16, n4 * 512:(n4 + 1) * 512]
                    )
                    wtiles[(bh, n4)] = [wtba, wtb2]

            # ---- phase 2: attention, per batch; A2A after each batch ----
            for b in range(B):
                qk_sb, v_sb = qk_sbs[b], v_sbs[b]
                for h in range(HPC):
                    for qc in reversed(range(L // 512)):
                        nk = 4 * qc + 4
                        outp = pp.tile([128, 512], f32, tag="acc", bufs=3,
                                       name=f"outp{b}_{h}_{qc}")
                        sump = pp.tile([1, 512], f32, tag="sump", bufs=2,
                                       name=f"sump{b}_{h}_{qc}")
                        for ki in range(nk):
                            # causal: for diagonal k-tiles only q-cols >= j*128
                            j = max(0, ki - qc * 4)
                            c0 = j * 128
                            npr = 512 - c0
                            qs = slice(qc * 512 + c0, (qc + 1) * 512)
                            sp = pp.tile([128, 512], f32, tag="work", bufs=3,
                                         name=f"sp{b}_{h}_{qc}_{ki}")
                            nc.tensor.matmul(
                                sp[:, 0:npr],
                                lhsT=qk_sb[:, 2 * h + 1, ki * 128:(ki + 1) * 128],
                                rhs=qk_sb[:, 2 * h, qs],
                                start=True, stop=True,
                            )
                            et = apool.tile([128, 512], f16, tag="et", bufs=4)
                            nc.scalar.activation(
                                et[:, 0:npr], sp[:, 0:npr], AFT.Exp, scale=SCALE
                            )
                            if ki >= qc * 4:
                                nc.vector.tensor_tensor(
                                    et[:, 0:128], et[:, 0:128],
                                    mask_sb[:, 0, 0:128], op=OP.mult,
                                )
                            nc.tensor.matmul(
                                outp[:, c0:512],
                                lhsT=v_sb[:, ki, h * 128:(h + 1) * 128],
                                rhs=et[:, 0:npr],
                                start=(ki == 0), stop=(ki == nk - 1),
                            )
                            nc.tensor.matmul(
                                sump[:, c0:512], lhsT=ones_c[:], rhs=et[:, 0:npr],
                                start=(ki == 0), stop=(ki == nk - 1),
                            )
                        osum = apool.tile([128, 512], f32, tag="osum", bufs=2)
                        nc.vector.tensor_copy(osum[:], outp[:])
                        rec = apool.tile([1, 512], f32, tag="rec", bufs=2)
                        nc.vector.reciprocal(rec[:], sump[:])
                        bcs = apool.tile([128, 512], f32, tag="bcs", bufs=2)
                        nc.gpsimd.partition_broadcast(bcs[:], rec[:])
                        ot = apool.tile([128, 512], f16, tag="ot", bufs=2)
                        nc.vector.tensor_tensor(ot[:], osum[:], bcs[:], op=OP.mult)
                        for half in range(2):
                            nc.sync.dma_start(
                                out=a2a_ins[b][2 * qc + half,
                                               h * 128:(h + 1) * 128, :],
                                in_=ot[:, half * 256:(half + 1) * 256],
                            )
                nc.gpsimd.collective_compute(
                    "AllToAll",
                    mybir.AluOpType.bypass,
                    replica_groups=[CORES],
                    ins=[a2a_ins[b][:]],
                    outs=[a2a_outs[b][:]],
                )
                # load the resharded half + rowsums, normalize in place
                # (gpsimd/DVE are idle here; b=0's pass overlaps b=1 attention)
                o_sb = p3pool.tile([128, 16, 256], f16, name=f"osb{b}")
                o_sb_v = o_sb[:].rearrange("p (j u) n -> p j u n", j=W)
                for u in range(2):
                    nc.gpsimd.dma_start(
                        out=o_sb_v[:, :, u, :],
                        in_=a2a_outs[b][:, u * 128:(u + 1) * 128, :].rearrange(
                            "j p n -> p j n"),
                    )
                o_sbs.append(o_sb)

            # ---- phase 3: output projection, b=0 half then b=1 half ----
            for bh in range(2):
                for n4 in range(4):
                    wparts = wtiles[(bh, n4)]
                    for i in range(2):
                        m = 2 * bh + i
                        yp = pp.tile([128, 512], f32,
                                     tag=("work" if i else "acc"), bufs=3,
                                     name=f"yps{n4}_{m}")
                        for kt in range(16):
                            nc.tensor.matmul(
                                yp[:],
                                lhsT=o_sbs[bh][:, kt, i * 128:(i + 1) * 128],
                                rhs=wparts[kt // 8][:, kt % 8, :],
                                start=(kt == 0), stop=(kt == 15),
                            )
                        yt = p3s.tile([128, 512], f32, tag="yt")
                        nc.vector.tensor_tensor(
                            yt[:], yp[:], bd_sb[:, n4 * 512:(n4 + 1) * 512],
                            op=OP.add,
                        )
                        nc.sync.dma_start(
                            out=y[m * 128:(m + 1) * 128,
                                  n4 * 512:(n4 + 1) * 512],
                            in_=yt[:],
                        )
    nc.finalize()
    return nc


def _host_prep(x_BLD, cos, sin, Wqkv, bqkv, Wd, bd):
    x = np.asarray(x_BLD, np.float32).reshape(M, D)
    xT = np.ascontiguousarray(x.T.astype(np.float16))
    c2 = np.asarray(cos, np.float32).reshape(L, R).T
    s2 = np.asarray(sin, np.float32).reshape(L, R).T
    cosT = np.ascontiguousarray(np.concatenate([c2] * B, axis=1))
    sinT = np.concatenate([s2] * B, axis=1)
    sinT_pm = np.ascontiguousarray(
        np.concatenate([-sinT[:16], sinT[16:]], axis=0)
    )
    kk = np.arange(128, dtype=np.int64)[:, None]
    qq = np.arange(512, dtype=np.int64)[None, :]
    masks = np.stack(
        [(qq >= j * 128 + kk).astype(np.float16) for j in range(4)]
    )
    bdb = np.ascontiguousarray(
        np.broadcast_to(np.asarray(bd, np.float32), (128, D))
    )
    Wqkv = np.asarray(Wqkv, np.float32)
    bqkv = np.asarray(bqkv, np.float32)
    in_maps = []
    for c in range(W):
        base = c * HPC * 3 * HD
        qk_idx = np.concatenate(
            [np.arange(base + h * 3 * HD, base + h * 3 * HD + 2 * HD)
             for h in range(HPC)]
        )
        v_idx = np.concatenate(
            [np.arange(base + h * 3 * HD + 2 * HD, base + (h + 1) * 3 * HD)
             for h in range(HPC)]
        )
        in_maps.append({
            "xT": xT,
            "wqkv": np.ascontiguousarray(
                Wqkv[:, np.concatenate([qk_idx, v_idx])].astype(np.float16)
            ),
            "bqk": np.ascontiguousarray(bqkv[qk_idx].reshape(4 * HD, 1)),
            "bv": np.ascontiguousarray(
                bqkv[v_idx].reshape(1, 2 * HD).astype(np.float16)
            ),
            "cosT": cosT.astype(np.float16),
            "sinT": sinT_pm.astype(np.float16),
            "masks": masks,
            "wd": np.asarray(Wd, np.float32).astype(np.float16),
            "bdb": bdb,
            "onesc": np.ones((128, 1), np.float16),
        })
    return in_maps


def _get_nc():
    global _NC
    if _NC is None:
        _NC = _build_nc()
    return _NC


def _run(inputs, trace=False, tmpdir=None):
    from concourse.bass_utils import run_bass_kernel_spmd

    in_maps = _host_prep(**inputs)
    nc = _get_nc()
    res = run_bass_kernel_spmd(nc, in_maps, CORES, trace=trace, tmpdir=tmpdir)
    out = np.empty((M, D), np.float32)
    for c in CORES:
        yc = res.results[c]["y"]          # [512, D]: rows b0 then b1
        out[c * 256:(c + 1) * 256] = yc[:256]
        out[L + c * 256:L + (c + 1) * 256] = yc[256:]
    return out.reshape(B, L, D), res


def kernel(**inputs) -> np.ndarray:
    out, _ = _run(inputs)
    return out
